# revision 13
# baseline (speedup 1.0000x reference)
"""Trainium2 Bass kernel for the 4-layer spiking (LIF) actor network.

Math (per layer, per timestep; carried states cur/volt/spike):
    cur_t  = 0.5*cur_{t-1} + z_t          z_t = pre_t @ W (+b, b==0 here)
    volt_t = 0.75*volt_{t-1}*(1-s_{t-1}) + cur_t
    s_t    = volt_t > 0.5
Re-parametrized (exact): vr_t := volt_t*(volt_t<=0.5) = min(volt_t,0.5)-0.5*s_t
    volt_t = cur_t + 0.75*vr_{t-1}

Device mapping (per core, B=512 batch rows, layout [feature->part, batch->free]):
  - PE: P(psum) = z + 0.5*Id@cur_{t-1}  (= cur_t).
        z matmuls in bf16 hi/lo pairs: spikes s"=0.5*s are EXACT in bf16;
        weights split w = wh + wl (both bf16) so products are exact to ~2^-18.
        x is split on host into xh+xl (bf16 pair, same DMA bytes as fp32);
        L1 uses the 3-pass xh@wh + xh@wl + xl@wh.
        The single cur identity-matmul stays fp32 (4 cyc/row on PE).
  - ScalarE: cur_t PSUM -> SBUF copy.
  - VectorE: volt = (vr_old * 0.75) + P        (scalar_tensor_tensor)
             s"   = (volt > 0.5) * 0.5 -> bf16 (dual-op tensor_scalar)
             vr   = min(volt, 0.5) - s"        (scalar_tensor_tensor)
  - Output sum_t(s4") accumulates in a PSUM bank via a bf16 2x2 Id matmul.

Sharding: data-parallel over batch across 8 cores; weights replicated.
"""
import sys

sys.path.insert(0, "/opt/trn_rl_repo")
import numpy as np

T, S, H, A = 50, 256, 256, 2
BC = 512  # batch rows per core
NCORES = 8
P = 128
KT = S // P
MT = H // P

_cache: dict = {}


def _build(nT=T, dump=0):
    from contextlib import ExitStack

    import concourse.tile as tile
    from concourse import bacc, mybir

    f32 = mybir.dt.float32
    bf16 = mybir.dt.bfloat16
    fp16 = mybir.dt.float16
    Alu = mybir.AluOpType

    nc = bacc.Bacc("TRN2", target_bir_lowering=False, debug=False, num_devices=NCORES)
    xhd = nc.dram_tensor("xTh", [T, S, BC], fp16, kind="ExternalInput").ap()
    xld = nc.dram_tensor("xTl", [T, S, BC], fp16, kind="ExternalInput").ap()
    w1hd = nc.dram_tensor("w1h", [S, H], fp16, kind="ExternalInput").ap()
    w1ld = nc.dram_tensor("w1l", [S, H], fp16, kind="ExternalInput").ap()
    wsd = {}
    for l, (rows, cols) in ((2, (H, H)), (3, (H, H)), (4, (H, A))):
        for nm in ("h", "l"):
            wsd[(l, nm)] = nc.dram_tensor(f"w{l}{nm}", [rows, cols], fp16,
                                          kind="ExternalInput").ap()
    id05d = nc.dram_tensor("id05", [P, P], f32, kind="ExternalInput").ap()
    id205d = nc.dram_tensor("id205", [A, A], f32, kind="ExternalInput").ap()
    id21d = nc.dram_tensor("id21", [A, A], fp16, kind="ExternalInput").ap()
    outd = nc.dram_tensor("out", [A, BC], f32, kind="ExternalOutput").ap()

    with tile.TileContext(nc) as tc, ExitStack() as ctx:
        consts = ctx.enter_context(tc.tile_pool(name="consts", bufs=1))
        xpool = ctx.enter_context(tc.tile_pool(name="xp", bufs=8))
        cpool = ctx.enter_context(tc.tile_pool(name="cur", bufs=2))
        vpool = ctx.enter_context(tc.tile_pool(name="volt", bufs=2))
        rpool = ctx.enter_context(tc.tile_pool(name="vr", bufs=2))
        spool = ctx.enter_context(tc.tile_pool(name="sp", bufs=2))
        ppool = ctx.enter_context(tc.tile_pool(name="psum", bufs=1, space="PSUM"))

        # ---- constants ----
        w1t = {}  # (hi/lo, k, m) -> [128,128] fp16 lhsT block
        for nm, dram in (("h", w1hd), ("l", w1ld)):
            for k in range(KT):
                for m in range(MT):
                    t_ = consts.tile([P, P], fp16, tag=f"w1{nm}{k}{m}")
                    nc.sync.dma_start(t_[:], dram[k * P:(k + 1) * P, m * P:(m + 1) * P])
                    w1t[(nm, k, m)] = t_
        wt = {}  # (layer, term, k, m) -> [128, 128] fp16 lhsT block
        for l in (2, 3):
            for k in range(KT):
                for m in range(MT):
                    for nm in ("h", "l"):
                        t_ = consts.tile([P, P], fp16, tag=f"w{l}{nm}{k}{m}")
                        nc.sync.dma_start(
                            t_[:], wsd[(l, nm)][k * P:(k + 1) * P, m * P:(m + 1) * P])
                        wt[(l, nm, k, m)] = t_
        w4t = {}
        for k in range(KT):
            for nm in ("h", "l"):
                t_ = consts.tile([P, A], fp16, tag=f"w4{nm}{k}")
                nc.sync.dma_start(t_[:], wsd[(4, nm)][k * P:(k + 1) * P, :])
                w4t[(nm, k)] = t_
        id05 = consts.tile([P, P], f32, tag="id05")
        nc.sync.dma_start(id05[:], id05d[:])
        id2_05 = consts.tile([A, A], f32, tag="id205")
        nc.sync.dma_start(id2_05[:], id205d[:])
        id2_1 = consts.tile([A, A], fp16, tag="id21")
        nc.sync.dma_start(id2_1[:], id21d[:])

        accp = ctx.enter_context(tc.tile_pool(name="accp", bufs=1, space="PSUM"))
        acc = accp.tile([A, BC], f32, tag="acc")


        # rotating state refs
        cur = {}   # key -> sbuf tile (fp32)
        vr = {}    # key -> sbuf tile (fp32), single history slot
        sp = {}    # li -> [m0, m1] bf16 spike tiles (s" = 0.5*s)
        volt = {}

        # zero-init vr tiles (consumed at t=0)
        for li in range(3):
            for m in range(MT):
                zt = rpool.tile([P, BC], f32, tag=f"vr{li}{m}")
                nc.vector.memset(zt[:], 0.0)
                vr[(li, m)] = zt
        z4 = rpool.tile([A, BC], f32, tag="vr3")
        nc.vector.memset(z4[:], 0.0)
        vr[(3, 0)] = z4

        C5SET = {(1, 0), (3, 0)}  # tiles whose cur-update runs on DVE

        def state_update(key, pt, li, m, t):
            """common post-matmul chain for one tile.
            C3 tiles: pt already holds cur_t (Id-MM added 0.5*cur_old); ACT
            copies it out.  C5 tiles: pt holds only z_t; DVE folds the decay."""
            nparts = P if li < 3 else A
            if key in C5SET:
                cnew = cpool.tile([nparts, BC], f32, tag=f"cur{li}{m}")
                if t > 0:
                    nc.vector.scalar_tensor_tensor(cnew[:], cur[key][:], 0.5,
                                                   pt[:], Alu.mult, Alu.add)
                else:
                    nc.vector.tensor_copy(cnew[:], pt[:])
                vnew = vpool.tile([nparts, BC], f32, tag=f"volt{li}{m}")
                nc.vector.scalar_tensor_tensor(vnew[:], vr[key][:], 0.75,
                                               cnew[:], Alu.mult, Alu.add)
            else:
                cnew = cpool.tile([nparts, BC], f32, tag=f"cur{li}{m}")
                nc.scalar.copy(cnew[:], pt[:])
                vnew = vpool.tile([nparts, BC], f32, tag=f"volt{li}{m}")
                nc.vector.scalar_tensor_tensor(vnew[:], vr[key][:], 0.75, cnew[:],
                                               Alu.mult, Alu.add)
            snew = spool.tile([nparts, BC], fp16, tag=f"sp{li}{m}")
            nc.vector.tensor_scalar(snew[:], vnew[:], 0.5, 0.5,
                                    Alu.is_gt, Alu.mult)
            rnew = rpool.tile([nparts, BC], f32, tag=f"vr{li}{m}")
            nc.vector.scalar_tensor_tensor(rnew[:], vnew[:], 0.5, snew[:],
                                           Alu.min, Alu.subtract)
            cur[key] = cnew
            vr[key] = rnew
            volt[key] = vnew
            return snew

        def cell(t, li):
            if li < 3:
                l = li + 1
                if l == 1:
                    rh_h, rh_l = [], []
                    for k in range(KT):
                        xt = xpool.tile([P, BC], fp16, tag="x")
                        nc.sync.dma_start(xt[:], xhd[t, k * P:(k + 1) * P, :])
                        rh_h.append(xt)
                        xt2 = xpool.tile([P, BC], fp16, tag="x")
                        nc.sync.dma_start(xt2[:], xld[t, k * P:(k + 1) * P, :])
                        rh_l.append(xt2)
                else:
                    rh_h = sp[li - 1]
                new_sp = []
                for m in range(MT):
                    pt = ppool.tile([P, BC], f32, tag=f"P{li}{m}")
                    mms = []
                    if t > 0 and (li, m) not in C5SET:
                        mms.append((id05, cur[(li, m)]))
                    for k in range(KT):
                        if l == 1:
                            mms.append((w1t[("h", k, m)], rh_h[k]))
                            mms.append((w1t[("l", k, m)], rh_h[k]))
                            mms.append((w1t[("h", k, m)], rh_l[k]))
                        else:
                            mms.append((wt[(l, "h", k, m)], rh_h[k]))
                            mms.append((wt[(l, "l", k, m)], rh_h[k]))
                    for i, (lh, rh) in enumerate(mms):
                        nc.tensor.matmul(pt[:], lh[:], rh[:], start=(i == 0),
                                         stop=(i == len(mms) - 1))
                    new_sp.append(state_update((li, m), pt, li, m, t))
                sp[li] = new_sp
            else:
                rhs = sp[2]
                pt = ppool.tile([A, BC], f32, tag="P4")
                mms = []
                if t > 0 and (3, 0) not in C5SET:
                    mms.append((id2_05, cur[(3, 0)]))
                for k in range(KT):
                    mms.append((w4t[("h", k)], rhs[k]))
                    mms.append((w4t[("l", k)], rhs[k]))
                for i, (lh, rh) in enumerate(mms):
                    nc.tensor.matmul(pt[:], lh[:], rh[:], start=(i == 0),
                                     stop=(i == len(mms) - 1))
                snew = state_update((3, 0), pt, 3, 0, t)
                nc.tensor.matmul(acc[:], id2_1[:], snew[:], start=(t == 0),
                                 stop=(t == nT - 1), skip_group_check=True)

        # descending li: consumers of sp[li-1] must run before cell(t+1, li-1)
        # replaces the python-side reference within the same diagonal
        for d in range(nT + 4):
            for li in (3, 2, 1, 0):
                t = d - li
                if 0 <= t < nT:
                    cell(t, li)
                    if dump and t < dump:
                        if li < 3:
                            for m in range(MT):
                                dt_ = nc.dram_tensor(f"dbg_v_{t}_{li}_{m}", [P, BC],
                                                     f32, kind="ExternalOutput").ap()
                                nc.sync.dma_start(dt_[:], volt[(li, m)][:])
                        else:
                            dt_ = nc.dram_tensor(f"dbg_v_{t}_3_0", [A, BC], f32,
                                                 kind="ExternalOutput").ap()
                            nc.sync.dma_start(dt_[:], volt[(3, 0)][:])

        # out = sum_t(s4)/T^2 = acc * 2 / T^2   (acc holds sum of 0.5*s4)
        ot = consts.tile([A, BC], f32, tag="ot")
        nc.scalar.mul(ot[:], acc[:], 2.0 / (T * T))
        nc.sync.dma_start(outd[:], ot[:])

    nc.compile()
    return nc


def _get_nc():
    if "nc" not in _cache:
        _cache["nc"] = _build()
    return _cache["nc"]


def _split_fp16_2(a):
    hi = np.ascontiguousarray(a.astype(np.float16))
    lo = np.ascontiguousarray((a - hi.astype(np.float32)).astype(np.float16))
    return hi, lo


def make_in_maps(x, w1, w2, w3, w4):
    """Host prep: shard x over batch, transpose to [T,S,Bc], split into bf16
    hi+lo; prescale w2..w4 by 2 (compensates s"=0.5*s) and split into bf16
    hi+lo pairs; identity blocks."""
    w = {2: 2.0 * np.float32(w2), 3: 2.0 * np.float32(w3),
         4: 2.0 * np.float32(w4)}
    base = {}
    base["w1h"], base["w1l"] = _split_fp16_2(np.float32(w1))
    for l in (2, 3, 4):
        base[f"w{l}h"], base[f"w{l}l"] = _split_fp16_2(w[l])
    base["id05"] = (0.5 * np.eye(P)).astype(np.float32)
    base["id205"] = (0.5 * np.eye(A)).astype(np.float32)
    base["id21"] = np.eye(A).astype(np.float16)
    in_maps = []
    for c in range(NCORES):
        xs = np.asarray(x[c * BC:(c + 1) * BC], np.float32)  # [BC, S, T]
        xTc = xs.transpose(2, 1, 0)                          # [T, S, BC]
        xh, xlo = _split_fp16_2(xTc)
        in_maps.append({"xTh": xh, "xTl": xlo, **base})
    return in_maps


def kernel(x, w1, b1, w2, b2, w3, b3, w4, b4, batch_size):
    from concourse.bass_utils import run_bass_kernel_spmd

    x = np.asarray(x)
    assert x.shape == (NCORES * BC, S, T), x.shape
    # biases are zero in this problem's setup; the kernel folds them out.
    for b in (b1, b2, b3, b4):
        assert np.all(np.asarray(b) == 0.0), "nonzero bias unsupported"
    nc = _get_nc()
    in_maps = make_in_maps(x, np.asarray(w1), np.asarray(w2), np.asarray(w3),
                           np.asarray(w4))
    res = run_bass_kernel_spmd(nc, in_maps, list(range(NCORES)))
    out = np.empty((NCORES * BC, A), np.float32)
    for c in range(NCORES):
        out[c * BC:(c + 1) * BC] = res.results[c]["out"].T
    return out
